# revision 2
# baseline (speedup 1.0000x reference)
"""Trainium2 Bass kernel for nn_Loss_Synonymy (v2).

Computes: sum over rows of relu(1 -/+ tanh(||S1_row - S2_row||_2)), the sign
chosen per-row by synonymy_score >= 0.6.

Strategy (pure data-parallel over 8 NeuronCores):
  - Shard the batch dim B=1048576 across 8 cores (131072 rows each).
  - Partition-major row ownership: partition p owns rows [p*1024,(p+1)*1024)
    of the core's shard, so the score vector for the whole shard is ONE fully
    contiguous [128, 1024] DMA (4 KiB per partition line).
  - Per slab s (32 slabs): stream [128, 4096] f32 tiles of S1/S2 (2 MiB DMAs,
    16 KiB contiguous per partition line). DVE subtract -> ACT square (in
    place) -> DVE segmented reduce over the innermost 128 gives per-row
    sum-of-squares into ss[:, s*32:(s+1)*32].
  - Epilogue on [128, 1024]: sqrt -> tanh -> clamp to 1.0; then
    w = (score < 0.6)*2, and one fused scalar_tensor_tensor
    (w - 1) * t with accum_out gives per-partition sums of sign*t.
  - Host: result = B + sum(all partials)   (since err = 1 + sign*t >= 0).

The reps loop (timing builds) repeats the FULL computation - score DMA,
streaming, epilogue, output DMA - so the reps-difference time measures one
complete logical execution, not just the streaming loop.
"""

import sys

if "/opt/trn_rl_repo" not in sys.path:
    sys.path.insert(0, "/opt/trn_rl_repo")

import numpy as np

B, D = 1048576, 128
NCORES = 8
BS = B // NCORES          # rows per core = 131072
P = 128                   # SBUF partitions
COLS = 4096               # free elems per slab
R = COLS // D             # rows per partition per slab = 32
NSLAB = BS // (P * R)     # slabs per core = 32
CPP = BS // P             # per-row values per partition = 1024
THRESH = 0.6

_nc_cache = {}


def _build_nc(reps=1, nslab=NSLAB, cols=COLS, loop=False, staggered=False, bodies=1):
    """Build the per-core Bass program. reps>1 repeats the full computation
    inside one NEFF (timing-measurement builds only); loop=True wraps the
    single body in a tc.For_i hardware loop instead of unrolling."""
    import concourse.bass as bass  # noqa: F401
    from concourse import bacc
    import concourse.tile as tile
    import concourse.mybir as mybir

    f32 = mybir.dt.float32
    rr = cols // D
    bs = nslab * P * rr
    cpp = bs // P
    nc = bacc.Bacc(None)
    s1 = nc.dram_tensor("s1", [bs, D], f32, kind="ExternalInput")
    s2 = nc.dram_tensor("s2", [bs, D], f32, kind="ExternalInput")
    sc = nc.dram_tensor("score", [bs], f32, kind="ExternalInput")
    out = nc.dram_tensor("out", [P, 1], f32, kind="ExternalOutput")

    with tile.TileContext(nc) as tc:
        with (
            tc.tile_pool(name="p1", bufs=3) as p1,
            tc.tile_pool(name="p2", bufs=3) as p2,
            tc.tile_pool(name="psq", bufs=3) as psq,
            tc.tile_pool(name="pss", bufs=2) as pss,
            tc.tile_pool(name="psc", bufs=2) as psc,
            tc.tile_pool(name="pw", bufs=2) as pw,
            tc.tile_pool(name="pacc", bufs=2) as pacc,
        ):
            # Partition-major: global row g = p*cpp + s*rr + r.
            s1v = s1[:].rearrange("(p s r) d -> s p (r d)", p=P, s=nslab, r=rr)
            s2v = s2[:].rearrange("(p s r) d -> s p (r d)", p=P, s=nslab, r=rr)
            scv = sc[:].rearrange("(p c) -> p c", p=P)

            def body():
                sct = psc.tile([P, cpp], f32)
                nc.sync.dma_start(sct[:], scv)
                ss = pss.tile([P, cpp], f32)
                for s in range(nslab):
                    t1 = p1.tile([P, cols], f32)
                    nc.sync.dma_start(t1[:], s1v[s])
                    t2 = p2.tile([P, cols], f32)
                    nc.sync.dma_start(t2[:], s2v[s])
                    sq = psq.tile([P, cols], f32)
                    nc.vector.tensor_sub(sq[:], t1[:], t2[:])
                    nc.scalar.square(sq[:], sq[:])
                    nc.vector.reduce_sum(
                        ss[:, s * rr:(s + 1) * rr],
                        sq[:].rearrange("p (r d) -> p r d", d=D),
                        axis=mybir.AxisListType.X,
                    )

                # dist = sqrt(ss); t = tanh(dist); clamp t <= 1.0 so that
                # relu(1 +/- t) == 1 +/- t exactly.
                nc.scalar.sqrt(ss[:], ss[:])
                nc.scalar.activation(
                    ss[:], ss[:], mybir.ActivationFunctionType.Tanh
                )
                nc.vector.tensor_scalar_min(ss[:], ss[:], 1.0)
                # w = (score < 0.6) * 2  in {0, 2}
                w = pw.tile([P, cpp], f32)
                nc.vector.tensor_scalar(
                    w[:], sct[:], THRESH, 2.0,
                    op0=mybir.AluOpType.is_lt, op1=mybir.AluOpType.mult,
                )
                # acc[p] = sum_c (w - 1) * t   (sign in {-1,+1})
                acc = pacc.tile([P, 1], f32)
                nc.vector.scalar_tensor_tensor(
                    w[:], w[:], -1.0, ss[:],
                    op0=mybir.AluOpType.add, op1=mybir.AluOpType.mult,
                    accum_out=acc[:],
                )
                nc.sync.dma_start(out[:], acc[:])

            if loop:
                with tc.For_i(0, reps, 1, staggered_reset=staggered):
                    for _b in range(bodies):
                        body()
            else:
                for _rep in range(reps):
                    body()
    nc.finalize()
    return nc


def _build_nc_loop(iters, staggered=True, bodies=4):
    return _build_nc(iters, loop=True, staggered=staggered, bodies=bodies)


def _get_nc(reps=1):
    if reps not in _nc_cache:
        _nc_cache[reps] = _build_nc(reps)
    return _nc_cache[reps]


def _in_maps(S1_out, S2_out, synonymy_score):
    s1 = np.ascontiguousarray(np.asarray(S1_out, dtype=np.float32))
    s2 = np.ascontiguousarray(np.asarray(S2_out, dtype=np.float32))
    sc = np.ascontiguousarray(np.asarray(synonymy_score, dtype=np.float32))
    assert s1.shape == (B, D) and s2.shape == (B, D) and sc.shape == (B,)
    return [
        {
            "s1": s1[c * BS:(c + 1) * BS],
            "s2": s2[c * BS:(c + 1) * BS],
            "score": sc[c * BS:(c + 1) * BS],
        }
        for c in range(NCORES)
    ]


def _postprocess(results):
    partials = np.concatenate([r["out"].ravel() for r in results])
    total = np.float64(B) + partials.astype(np.float64).sum()
    return np.float32(total)


def kernel(S1_out, S2_out, synonymy_score):
    from concourse.bass_utils import run_bass_kernel_spmd

    in_maps = _in_maps(S1_out, S2_out, synonymy_score)
    res = run_bass_kernel_spmd(_get_nc(), in_maps, list(range(NCORES)))
    return _postprocess(res.results)
